# revision 1
# baseline (speedup 1.0000x reference)
"""Bidirectional ConvLSTM block for Trainium2 (Bass/Tile), 8-core SPMD.

Problem: x [S=16, B=4, Cin=32, H=128, W=128] f32, Wf/Wb [128, 64, 3, 3],
bf/bb [128].  Output [S, B, 2*Co=64, H, W]: forward ConvLSTM hidden states
concat backward ConvLSTM (run on time-reversed x, not re-flipped).

Sharding: 8 independent recurrences = 2 directions x 4 batch elements.
Core k runs direction d=k//4 on batch b=k%4.  No cross-core communication.

Per-core kernel design:
  - SBUF "act" tile [128 part, 130*130] bf16 (ping/pong): zero-padded
    (H+2)x(W+2) spatial plane per channel.
      partitions  0-31 : x_t   (center copy)
      partitions 32-63 : h_{t-1} (center copy)
      partitions 64-95 : x_t   shifted right by one column (value of left nbr)
      partitions 96-127: h_{t-1} shifted
    The shifted copy lets one K=128 matmul cover two conv taps (dx=0 via
    center rows, dx=-1 via shifted rows) at a single rhs column offset.
    3x3 conv => 6 matmul passes per output tile: 3 passes pair
    (dy,0)+(dy,-1); 3 passes do (dy,+1) with zero weights on shifted rows.
  - Col-tiled matmuls (tile_position (0,32j), mode 128x32): 4 concurrent
    M=32 matmuls per pass, one per spatial tile, each writing ITS gate's
    slice so each PSUM bank ends up gate-pure:
      bank[g] = [128 part = 32ch x 4 spatial tiles, 512] of gate g.
    All pointwise LSTM math then runs on full 128-partition tiles.
  - c state persistent fp32 [128, 4096]; h written back (fp32 -> HBM out,
    bf16 -> next step's act tile via SBUF-SBUF DMA with layout shuffle).
"""

import os
import sys

import numpy as np

for _p in ("/opt/trn_rl_repo", "/root/.axon_site/_ro/trn_rl_repo"):
    if os.path.isdir(_p) and _p not in sys.path:
        sys.path.insert(0, _p)

import ml_dtypes  # noqa: E402
import concourse.bass as bass  # noqa: E402,F401
import concourse.mybir as mybir  # noqa: E402
from concourse import bacc, tile  # noqa: E402
from concourse.bass_utils import run_bass_kernel_spmd  # noqa: E402

F32 = mybir.dt.float32
BF16 = mybir.dt.bfloat16
AF = mybir.ActivationFunctionType

S, B, CIN, H, W = 16, 4, 32, 128, 128
CO = 32
HP, WP = H + 2, W + 2          # 130 x 130 padded plane
PADN = HP * WP                 # 16900
NSP = H * W                    # 16384
NT = 512                       # spatial positions per matmul tile (4 rows)
TPG = 4                        # tiles per group (col-tiled together)
GROUPS = NSP // (NT * TPG)     # 8 groups per step; group = 16 image rows
N_CORES = 8


def build_kernel(nc, tc, x_ap, w_ap, b_ap, y_ap, n_steps):
    ctx_pools = []

    def pool(**kw):
        p = tc.tile_pool(**kw)
        ctx_pools.append(p)
        return p.__enter__()

    const = pool(name="const", bufs=1)
    tmp = pool(name="tmp", bufs=3)
    psum = pool(name="psum", bufs=8, space="PSUM")

    # Persistent tiles
    a0 = const.tile([128, PADN], BF16, tag="act0")
    a1 = const.tile([128, PADN], BF16, tag="act1")
    acts = [a0, a1]
    ctile = const.tile([128, GROUPS * NT], F32, tag="c")
    wsb = const.tile([128, 24 * 32], BF16, tag="w")
    bsb = const.tile([128, 4], F32, tag="bias")

    nc.sync.dma_start(wsb[:, :], w_ap)
    nc.sync.dma_start(bsb[:, :], b_ap)

    # --- one-time zero init ---
    for a in acts:
        ar = a.rearrange("p (r w) -> p r w", r=HP)
        nc.gpsimd.memset(a[:, 0:WP], 0.0)                       # pad row 0
        nc.gpsimd.memset(a[:, (HP - 1) * WP :], 0.0)            # pad row 129
        nc.gpsimd.memset(ar[:, :, 0:1], 0.0)                    # col 0 (unread, keep finite)
        nc.gpsimd.memset(ar[0:64, :, WP - 1 : WP], 0.0)         # center col 129
        nc.gpsimd.memset(ar[64:128, :, 1:2], 0.0)               # shift col 1
    # h regions of step-0 act buffer (h_0 = 0)
    nc.vector.memset(a0[32:64, :], 0.0)
    nc.gpsimd.memset(a0[96:128, :], 0.0)
    nc.vector.memset(ctile[:, :], 0.0)

    def load_x(t):
        # x arrives host-padded to the full 130x130 plane: both the center
        # copy and the +1-shifted copy are single contiguous runs per
        # partition (the shift picks up its zero border from the host pad).
        nc.sync.dma_start(acts[t % 2][0:32, :], x_ap[t])
        nc.sync.dma_start(acts[t % 2][64:96, 1:PADN], x_ap[t, :, 0 : PADN - 1])

    load_x(0)

    for t in range(n_steps):
        a_cur = acts[t % 2]
        a_nxt = acts[(t + 1) % 2]
        ar_cur = a_cur.rearrange("p (r w) -> p r w", r=HP)
        if t + 1 < n_steps:
            load_x(t + 1)

        yg = y_ap[t].rearrange("c (g b f) -> g b c f", b=TPG, f=NT)

        def pointwise(grp, zb):
            csl = ctile[:, grp * NT : (grp + 1) * NT]
            si = tmp.tile([128, NT], F32, tag="si", name=f"si{t}_{grp}")
            sf = tmp.tile([128, NT], F32, tag="sf", name=f"sf{t}_{grp}")
            so = tmp.tile([128, NT], F32, tag="so", name=f"so{t}_{grp}")
            tg = tmp.tile([128, NT], F32, tag="tg", name=f"tg{t}_{grp}")
            nc.scalar.activation(si[:, :], zb[0][:, :], AF.Sigmoid, bias=bsb[:, 0:1])
            nc.scalar.activation(sf[:, :], zb[1][:, :], AF.Sigmoid, bias=bsb[:, 1:2])
            nc.scalar.activation(so[:, :], zb[2][:, :], AF.Sigmoid, bias=bsb[:, 2:3])
            nc.scalar.activation(tg[:, :], zb[3][:, :], AF.Tanh, bias=bsb[:, 3:4])

            t2 = tmp.tile([128, NT], F32, tag="t2", name=f"t2_{t}_{grp}")
            t3 = tmp.tile([128, NT], F32, tag="t3", name=f"t3_{t}_{grp}")
            # t3 only needs sf (2nd activation) - run it before t2 so the
            # DVE isn't blocked behind tanh_g on the critical chain
            nc.vector.tensor_mul(t3[:, :], sf[:, :], csl)
            nc.vector.tensor_mul(t2[:, :], si[:, :], tg[:, :])
            nc.vector.tensor_add(csl, t2[:, :], t3[:, :])

            tcn = tmp.tile([128, NT], F32, tag="tcn", name=f"tcn{t}_{grp}")
            nc.scalar.activation(tcn[:, :], csl, AF.Tanh)
            h32 = tmp.tile([128, NT], F32, tag="h32", name=f"h32_{t}_{grp}")
            if t + 1 < n_steps:
                # bf16 recurrence value straight from the mul: the cast drops
                # off the h-rec critical path (y waits on h32, but y is slack)
                hbf = tmp.tile([128, NT], BF16, tag="hbf", name=f"hbf{t}_{grp}")
                nc.vector.tensor_mul(hbf[:, :], so[:, :], tcn[:, :])
            nc.vector.tensor_mul(h32[:, :], so[:, :], tcn[:, :])

            # output: per-spatial-tile DMAs (partition-split src APs mislower)
            for bb_ in range(TPG):
                src_b = h32[32 * bb_ : 32 * bb_ + 32, :]
                nc.sync.dma_start(yg[grp, bb_], src_b)

            if t + 1 < n_steps:
                ar_hc = a_nxt[32:64, :].rearrange("p (r w) -> p r w", r=HP)
                ar_hs = a_nxt[96:128, :].rearrange("p (r w) -> p r w", r=HP)
                for bb_ in range(TPG):
                    src_b = hbf[32 * bb_ : 32 * bb_ + 32, :].rearrange(
                        "c (r w) -> c r w", r=4
                    )
                    rr = 16 * grp + 4 * bb_ + 1
                    nc.scalar.dma_start(ar_hc[:, rr : rr + 4, 1 : W + 1], src_b)
                    nc.sync.dma_start(ar_hs[:, rr : rr + 4, 2 : W + 2], src_b)

        # Group pairs: adjacent groups' matmuls interleaved so same-weight
        # matmuls are consecutive (better PE pipelining).
        for pg in range(GROUPS // 2):
            grps = (2 * pg, 2 * pg + 1)
            zbs = [
                [
                    psum.tile([128, NT], F32, tag="z", name=f"z{t}_{grp}_{g}")
                    for g in range(4)
                ]
                for grp in grps
            ]
            for g in range(4):
                for p in range(6):
                    dy = (p % 3) - 1
                    dxo = 0 if p < 3 else 1
                    col = (g * 6 + p) * 32
                    lhsT = wsb[:, col : col + 32]
                    for j in range(TPG):
                        for gi, grp in enumerate(grps):
                            r0 = 16 * grp + 4 * j
                            rhs = ar_cur[
                                :, r0 + 1 + dy : r0 + 5 + dy, 1 + dxo : W + 1 + dxo
                            ]
                            nc.tensor.matmul(
                                zbs[gi][g][32 * j : 32 * j + 32, :],
                                lhsT,
                                rhs,
                                start=(p == 0),
                                stop=(p == 5),
                                skip_group_check=True,
                                tile_position=(0, 32 * j),
                            )
            pointwise(grps[0], zbs[0])
            pointwise(grps[1], zbs[1])

    for p in reversed(ctx_pools):
        p.__exit__(None, None, None)


def build_program(n_steps=S):
    nc = bacc.Bacc(
        "TRN2",
        target_bir_lowering=False,
        debug=False,
        enable_asserts=False,
        num_devices=N_CORES,
    )
    x_d = nc.dram_tensor("x", [n_steps, CIN, PADN], BF16, kind="ExternalInput")
    w_d = nc.dram_tensor("w", [128, 24 * 32], BF16, kind="ExternalInput")
    b_d = nc.dram_tensor("bias", [128, 4], F32, kind="ExternalInput")
    y_d = nc.dram_tensor("y", [n_steps, CO, NSP], F32, kind="ExternalOutput")
    with tile.TileContext(nc) as tc:
        build_kernel(nc, tc, x_d.ap(), w_d.ap(), b_d.ap(), y_d.ap(), n_steps)
    nc.compile()
    return nc


def pack_weights(Wd):
    """Wd [128, 64, 3, 3] f32 -> lhsT blocks [128, 24*32] bf16."""
    wp = np.zeros((128, 24, 32), np.float32)
    for g in range(4):
        Wg = Wd[g * 32 : (g + 1) * 32]  # [32(m), 64, 3, 3]
        for p in range(6):
            ky = (p % 3)  # dy + 1
            blk = wp[:, g * 6 + p, :]
            if p < 3:
                blk[0:32, :] = Wg[:, 0:32, ky, 1].T    # x, dx=0 via center rows
                blk[32:64, :] = Wg[:, 32:64, ky, 1].T  # h, dx=0
                blk[64:96, :] = Wg[:, 0:32, ky, 0].T   # x, dx=-1 via shifted rows
                blk[96:128, :] = Wg[:, 32:64, ky, 0].T
            else:
                blk[0:32, :] = Wg[:, 0:32, ky, 2].T    # x, dx=+1 via center rows
                blk[32:64, :] = Wg[:, 32:64, ky, 2].T
    return wp.reshape(128, 24 * 32).astype(ml_dtypes.bfloat16)


def pack_bias(bd):
    """bd [128] f32 -> [128, 4] f32 (partition p = 32*tile + ch)."""
    bp = np.zeros((128, 4), np.float32)
    for g in range(4):
        bp[:, g] = np.tile(bd[g * 32 : (g + 1) * 32], 4)
    return bp


def make_in_maps(x, Wf, bf, Wb, bb, n_steps=S):
    wpacks = [pack_weights(np.asarray(Wf, np.float32)),
              pack_weights(np.asarray(Wb, np.float32))]
    bpacks = [pack_bias(np.asarray(bf, np.float32)),
              pack_bias(np.asarray(bb, np.float32))]
    x = np.asarray(x, np.float32)
    in_maps = []
    for k in range(N_CORES):
        d, b = k // 4, k % 4
        xc = x[:n_steps, b] if d == 0 else x[::-1][:n_steps, b]
        xp = np.zeros((n_steps, CIN, HP, WP), ml_dtypes.bfloat16)
        xp[:, :, 1 : H + 1, 1 : W + 1] = xc
        in_maps.append(
            {
                "x": xp.reshape(n_steps, CIN, PADN),
                "w": wpacks[d],
                "bias": bpacks[d],
            }
        )
    return in_maps


_CACHED_NC = None


def kernel(x, Wf, bf, Wb, bb):
    global _CACHED_NC
    if _CACHED_NC is None:
        _CACHED_NC = build_program(S)
    nc = _CACHED_NC
    in_maps = make_in_maps(x, Wf, bf, Wb, bb)
    res = run_bass_kernel_spmd(nc, in_maps, core_ids=list(range(N_CORES)))
    out = np.empty((S, B, 2 * CO, H, W), np.float32)
    for k in range(N_CORES):
        d, b = k // 4, k % 4
        yk = res.results[k]["y"].reshape(S, CO, H, W)
        out[:, b, d * CO : (d + 1) * CO] = yk
    return out


if __name__ == "__main__":
    import jax

    jax.config.update("jax_platforms", "cpu")
    rng = np.random.default_rng(0)
    x = rng.standard_normal((S, B, CIN, H, W), np.float32)
    Wf = (rng.standard_normal((128, 64, 3, 3)) * 0.05).astype(np.float32)
    Wb = (rng.standard_normal((128, 64, 3, 3)) * 0.05).astype(np.float32)
    bf = np.zeros(128, np.float32)
    bb = np.zeros(128, np.float32)
    y = kernel(x, Wf, bf, Wb, bb)
    print("out", y.shape, y.dtype)



# revision 13
# speedup vs baseline: 1.2661x; 1.2661x over previous
"""Bidirectional ConvLSTM block for Trainium2 (Bass/Tile), 8-core SPMD.

Problem: x [S=16, B=4, Cin=32, H=128, W=128] f32, Wf/Wb [128, 64, 3, 3],
bf/bb [128].  Output [S, B, 2*Co=64, H, W]: forward ConvLSTM hidden states
concat backward ConvLSTM (run on time-reversed x, not re-flipped).

Sharding: 8 independent recurrences = 2 directions x 4 batch elements.
Core k runs direction d=k//4 on batch b=k%4.  No cross-core communication.

Per-core kernel design:
  - SBUF "act" tile [128 part, 130*130] bf16 (ping/pong): zero-padded
    (H+2)x(W+2) spatial plane per channel.
      partitions  0-31 : x_t   (center copy)
      partitions 32-63 : h_{t-1} (center copy)
      partitions 64-95 : x_t   shifted right by one column (value of left nbr)
      partitions 96-127: h_{t-1} shifted
    The shifted copy lets one K=128 matmul cover two conv taps (dx=0 via
    center rows, dx=-1 via shifted rows) at a single rhs column offset.
    3x3 conv => 6 matmul passes per output tile: 3 passes pair
    (dy,0)+(dy,-1); 3 passes do (dy,+1) with zero weights on shifted rows.
  - Col-tiled matmuls (tile_position (0,32j), mode 128x32): 4 concurrent
    M=32 matmuls per pass, one per spatial tile, each writing ITS gate's
    slice so each PSUM bank ends up gate-pure:
      bank[g] = [128 part = 32ch x 4 spatial tiles, 512] of gate g.
    All pointwise LSTM math then runs on full 128-partition tiles.
  - c state persistent fp32 [128, 4096]; h written back (fp32 -> HBM out,
    bf16 -> next step's act tile via SBUF-SBUF DMA with layout shuffle).
"""

import os
import sys

import numpy as np

for _p in ("/opt/trn_rl_repo", "/root/.axon_site/_ro/trn_rl_repo"):
    if os.path.isdir(_p) and _p not in sys.path:
        sys.path.insert(0, _p)

import ml_dtypes  # noqa: E402
import concourse.bass as bass  # noqa: E402,F401
import concourse.mybir as mybir  # noqa: E402
from concourse import bacc, tile  # noqa: E402
from concourse.bass_utils import run_bass_kernel_spmd  # noqa: E402

F32 = mybir.dt.float32
BF16 = mybir.dt.bfloat16
AF = mybir.ActivationFunctionType

S, B, CIN, H, W = 16, 4, 32, 128, 128
CO = 32
HP, WP = H + 2, W + 2          # 130 x 130 padded plane
PADN = HP * WP                 # 16900
NSP = H * W                    # 16384
NT = 512                       # spatial positions per matmul tile (4 rows)
TPG = 4                        # tiles per group (col-tiled together)
GROUPS = NSP // (NT * TPG)     # 8 groups per step; group = 16 image rows
N_CORES = 8


def build_kernel(nc, tc, x_ap, w_ap, b_ap, y_ap, n_steps):
    ctx_pools = []

    def pool(**kw):
        p = tc.tile_pool(**kw)
        ctx_pools.append(p)
        return p.__enter__()

    const = pool(name="const", bufs=1)
    tmp = pool(name="tmp", bufs=3)
    psum = pool(name="psum", bufs=8, space="PSUM")

    # Persistent tiles
    a0 = const.tile([128, PADN], BF16, tag="act0")
    a1 = const.tile([128, PADN], BF16, tag="act1")
    acts = [a0, a1]
    ctile = const.tile([128, GROUPS * NT], F32, tag="c")
    wsb = const.tile([128, 24 * 32], BF16, tag="w")
    bsb = const.tile([128, 4], F32, tag="bias")
    # per-step h (bf16) in padded row-chunks of 130: partition = 32*tile+ch,
    # free = 8 groups x 4 rows x 130.  hc chunk = [0, w0..w127, 0] (center
    # alignment), hs chunk = [0, 0, w0..w127] (shift alignment).  Pads are
    # zeroed once; whole 4-row blocks are then contiguous 520-elem runs, so
    # the write-back into the padded act planes is a 3-dim DMA per tile b.
    NTP = 4 * WP  # 520
    hc = const.tile([128, GROUPS * NTP], BF16, tag="hc")
    hs = const.tile([128, GROUPS * NTP], BF16, tag="hs")

    nc.sync.dma_start(wsb[:, :], w_ap)
    nc.sync.dma_start(bsb[:, :], b_ap)

    # --- one-time zero init ---
    for a in acts:
        ar = a.rearrange("p (r w) -> p r w", r=HP)
        nc.gpsimd.memset(a[:, 0:WP], 0.0)                       # pad row 0
        nc.gpsimd.memset(a[:, (HP - 1) * WP :], 0.0)            # pad row 129
        nc.gpsimd.memset(ar[:, :, 0:1], 0.0)                    # col 0 (unread, keep finite)
        nc.gpsimd.memset(ar[0:64, :, WP - 1 : WP], 0.0)         # center col 129
        nc.gpsimd.memset(ar[64:128, :, 1:2], 0.0)               # shift col 1
    # h regions of step-0 act buffer (h_0 = 0)
    nc.vector.memset(a0[32:64, :], 0.0)
    nc.gpsimd.memset(a0[96:128, :], 0.0)
    nc.vector.memset(ctile[:, :], 0.0)

    def load_x(t):
        # x arrives host-padded to the full 130x130 plane: both the center
        # copy and the +1-shifted copy are single contiguous runs per
        # partition (the shift picks up its zero border from the host pad).
        nc.sync.dma_start(acts[t % 2][0:32, :], x_ap[t])
        nc.sync.dma_start(acts[t % 2][64:96, 1:PADN], x_ap[t, :, 0 : PADN - 1])

    load_x(0)
    nc.vector.memset(hc[:, :], 0.0)
    nc.gpsimd.memset(hs[:, :], 0.0)

    # h write-back: 4-row full-width (130) blocks, one 3-dim DMA per tile b.
    # Block (g, b) lands at padded plane rows 16g+4b+1 .. +5, cols 0..130.
    hcv = hc.rearrange("p (g f) -> p g f", g=GROUPS)
    hsv = hs.rearrange("p (g f) -> p g f", g=GROUPS)
    hcq = hc.rearrange("p (g q w) -> p g q w", g=GROUPS, q=4)
    hsq = hs.rearrange("p (g q w) -> p g q w", g=GROUPS, q=4)

    def h_writeback(a_nxt, g0, g1):
        # interior rows 1..129 of the padded plane = 32 blocks of (4 rows x 130)
        vc = a_nxt[32:64, WP : WP + H * WP].rearrange(
            "p (g b f) -> p g b f", g=GROUPS, b=TPG
        )
        vs = a_nxt[96:128, WP : WP + H * WP].rearrange(
            "p (g b f) -> p g b f", g=GROUPS, b=TPG
        )
        for b in range(TPG):
            nc.sync.dma_start(vc[:, g0:g1, b, :], hcv[32 * b : 32 * b + 32, g0:g1, :])
            nc.sync.dma_start(vs[:, g0:g1, b, :], hsv[32 * b : 32 * b + 32, g0:g1, :])

    for t in range(n_steps):
        a_cur = acts[t % 2]
        a_nxt = acts[(t + 1) % 2]
        ar_cur = a_cur.rearrange("p (r w) -> p r w", r=HP)
        if t + 1 < n_steps:
            load_x(t + 1)

        def pointwise(grp, zb):
            csl = ctile[:, grp * NT : (grp + 1) * NT]
            si = tmp.tile([128, NT], F32, tag="si", name=f"si{t}_{grp}")
            sf = tmp.tile([128, NT], F32, tag="sf", name=f"sf{t}_{grp}")
            so = tmp.tile([128, NT], F32, tag="so", name=f"so{t}_{grp}")
            tg = tmp.tile([128, NT], F32, tag="tg", name=f"tg{t}_{grp}")
            nc.scalar.activation(si[:, :], zb[0][:, :], AF.Sigmoid, bias=bsb[:, 0:1])
            nc.scalar.activation(sf[:, :], zb[1][:, :], AF.Sigmoid, bias=bsb[:, 1:2])
            nc.scalar.activation(so[:, :], zb[2][:, :], AF.Sigmoid, bias=bsb[:, 2:3])
            nc.scalar.activation(tg[:, :], zb[3][:, :], AF.Tanh, bias=bsb[:, 3:4])

            t2 = tmp.tile([128, NT], F32, tag="t2", name=f"t2_{t}_{grp}")
            t3 = tmp.tile([128, NT], F32, tag="t3", name=f"t3_{t}_{grp}")
            # t3 only needs sf (2nd activation) - run it before t2 so the
            # DVE isn't blocked behind tanh_g on the critical chain
            nc.vector.tensor_mul(t3[:, :], sf[:, :], csl)
            nc.vector.tensor_mul(t2[:, :], si[:, :], tg[:, :])
            nc.vector.tensor_add(csl, t2[:, :], t3[:, :])

            tcn = tmp.tile([128, NT], F32, tag="tcn", name=f"tcn{t}_{grp}")
            nc.scalar.activation(tcn[:, :], csl, AF.Tanh)
            # h in bf16, written into the padded row-chunk buffers: hc serves
            # the y store + center write-back, hs the shifted write-back.
            soq = so[:, :].rearrange("p (q w) -> p q w", q=4)
            tcq = tcn[:, :].rearrange("p (q w) -> p q w", q=4)
            nc.vector.tensor_mul(hcq[:, grp, :, 1 : W + 1], soq, tcq)
            if t + 1 < n_steps:
                nc.vector.tensor_mul(hsq[:, grp, :, 2 : W + 2], soq, tcq)

        # Group pairs: adjacent groups' matmuls interleaved so same-weight
        # matmuls are consecutive (better PE pipelining).
        for pg in range(GROUPS // 2):
            grps = (2 * pg, 2 * pg + 1)
            zbs = [
                [
                    psum.tile([128, NT], F32, tag="z", name=f"z{t}_{grp}_{g}")
                    for g in range(4)
                ]
                for grp in grps
            ]
            for g in range(4):
                for p in range(6):
                    dy = (p % 3) - 1
                    dxo = 0 if p < 3 else 1
                    col = (g * 6 + p) * 32
                    lhsT = wsb[:, col : col + 32]
                    for j in range(TPG):
                        for gi, grp in enumerate(grps):
                            r0 = 16 * grp + 4 * j
                            rhs = ar_cur[
                                :, r0 + 1 + dy : r0 + 5 + dy, 1 + dxo : W + 1 + dxo
                            ]
                            nc.tensor.matmul(
                                zbs[gi][g][32 * j : 32 * j + 32, :],
                                lhsT,
                                rhs,
                                start=(p == 0),
                                stop=(p == 5),
                                skip_group_check=True,
                                tile_position=(0, 32 * j),
                            )
            pointwise(grps[0], zbs[0])
            pointwise(grps[1], zbs[1])
            if pg == 1:
                nc.sync.dma_start(y_ap[t, :, 0 : 4 * NTP], hc[:, 0 : 4 * NTP])
            elif pg == 2 and t + 1 < n_steps:
                h_writeback(a_nxt, 0, 5)
            elif pg == 3:
                nc.sync.dma_start(y_ap[t, :, 4 * NTP :], hc[:, 4 * NTP :])
                if t + 1 < n_steps:
                    h_writeback(a_nxt, 5, 8)

    for p in reversed(ctx_pools):
        p.__exit__(None, None, None)


def build_program(n_steps=S):
    nc = bacc.Bacc(
        "TRN2",
        target_bir_lowering=False,
        debug=False,
        enable_asserts=False,
        num_devices=N_CORES,
    )
    x_d = nc.dram_tensor("x", [n_steps, CIN, PADN], BF16, kind="ExternalInput")
    w_d = nc.dram_tensor("w", [128, 24 * 32], BF16, kind="ExternalInput")
    b_d = nc.dram_tensor("bias", [128, 4], F32, kind="ExternalInput")
    # y in padded raw layout: [t, 32*tile+ch, group*520 + 130*q + (1+w)], bf16
    y_d = nc.dram_tensor(
        "y", [n_steps, 128, GROUPS * 4 * WP], BF16, kind="ExternalOutput"
    )
    with tile.TileContext(nc) as tc:
        build_kernel(nc, tc, x_d.ap(), w_d.ap(), b_d.ap(), y_d.ap(), n_steps)
    nc.compile()
    return nc


def pack_weights(Wd):
    """Wd [128, 64, 3, 3] f32 -> lhsT blocks [128, 24*32] bf16."""
    wp = np.zeros((128, 24, 32), np.float32)
    for g in range(4):
        Wg = Wd[g * 32 : (g + 1) * 32]  # [32(m), 64, 3, 3]
        for p in range(6):
            ky = (p % 3)  # dy + 1
            blk = wp[:, g * 6 + p, :]
            if p < 3:
                blk[0:32, :] = Wg[:, 0:32, ky, 1].T    # x, dx=0 via center rows
                blk[32:64, :] = Wg[:, 32:64, ky, 1].T  # h, dx=0
                blk[64:96, :] = Wg[:, 0:32, ky, 0].T   # x, dx=-1 via shifted rows
                blk[96:128, :] = Wg[:, 32:64, ky, 0].T
            else:
                blk[0:32, :] = Wg[:, 0:32, ky, 2].T    # x, dx=+1 via center rows
                blk[32:64, :] = Wg[:, 32:64, ky, 2].T
    return wp.reshape(128, 24 * 32).astype(ml_dtypes.bfloat16)


def pack_bias(bd):
    """bd [128] f32 -> [128, 4] f32 (partition p = 32*tile + ch)."""
    bp = np.zeros((128, 4), np.float32)
    for g in range(4):
        bp[:, g] = np.tile(bd[g * 32 : (g + 1) * 32], 4)
    return bp


def make_in_maps(x, Wf, bf, Wb, bb, n_steps=S):
    wpacks = [pack_weights(np.asarray(Wf, np.float32)),
              pack_weights(np.asarray(Wb, np.float32))]
    bpacks = [pack_bias(np.asarray(bf, np.float32)),
              pack_bias(np.asarray(bb, np.float32))]
    x = np.asarray(x, np.float32)
    in_maps = []
    for k in range(N_CORES):
        d, b = k // 4, k % 4
        xc = x[:n_steps, b] if d == 0 else x[::-1][:n_steps, b]
        xp = np.zeros((n_steps, CIN, HP, WP), ml_dtypes.bfloat16)
        xp[:, :, 1 : H + 1, 1 : W + 1] = xc
        in_maps.append(
            {
                "x": xp.reshape(n_steps, CIN, PADN),
                "w": wpacks[d],
                "bias": bpacks[d],
            }
        )
    return in_maps


_CACHED_NC = None


def unpack_y(yk):
    """[S, 128, 8*4*130] bf16 padded raw layout -> [S, CO, H, W] f32."""
    yk = np.asarray(yk, np.float32).reshape(S, TPG, CO, GROUPS, 4, WP)[..., 1 : W + 1]
    return np.ascontiguousarray(yk.transpose(0, 2, 3, 1, 4, 5)).reshape(S, CO, H, W)


def kernel(x, Wf, bf, Wb, bb):
    global _CACHED_NC
    if _CACHED_NC is None:
        _CACHED_NC = build_program(S)
    nc = _CACHED_NC
    in_maps = make_in_maps(x, Wf, bf, Wb, bb)
    res = run_bass_kernel_spmd(nc, in_maps, core_ids=list(range(N_CORES)))
    out = np.empty((S, B, 2 * CO, H, W), np.float32)
    for k in range(N_CORES):
        d, b = k // 4, k % 4
        out[:, b, d * CO : (d + 1) * CO] = unpack_y(res.results[k]["y"])
    return out


if __name__ == "__main__":
    import jax

    jax.config.update("jax_platforms", "cpu")
    rng = np.random.default_rng(0)
    x = rng.standard_normal((S, B, CIN, H, W), np.float32)
    Wf = (rng.standard_normal((128, 64, 3, 3)) * 0.05).astype(np.float32)
    Wb = (rng.standard_normal((128, 64, 3, 3)) * 0.05).astype(np.float32)
    bf = np.zeros(128, np.float32)
    bb = np.zeros(128, np.float32)
    y = kernel(x, Wf, bf, Wb, bb)
    print("out", y.shape, y.dtype)



# revision 21
# speedup vs baseline: 1.4586x; 1.1520x over previous
"""Bidirectional ConvLSTM block for Trainium2 (Bass/Tile), 8-core SPMD.

Problem: x [S=16, B=4, Cin=32, H=128, W=128] f32, Wf/Wb [128, 64, 3, 3],
bf/bb [128].  Output [S, B, 2*Co=64, H, W]: forward ConvLSTM hidden states
concat backward ConvLSTM (run on time-reversed x, not re-flipped).

Sharding: 8 independent recurrences = 2 directions x 4 batch elements.
Core k runs direction d=k//4 on batch b=k%4.  No cross-core communication.

Per-core kernel design:
  - SBUF "act" tile [128 part, 130*130] bf16 (ping/pong): zero-padded
    (H+2)x(W+2) spatial plane per channel.
      partitions  0-31 : x_t   (center copy)
      partitions 32-63 : h_{t-1} (center copy)
      partitions 64-95 : x_t   shifted right by one column (value of left nbr)
      partitions 96-127: h_{t-1} shifted
    The shifted copy lets one K=128 matmul cover two conv taps (dx=0 via
    center rows, dx=-1 via shifted rows) at a single rhs column offset.
    3x3 conv => 6 matmul passes per output tile: 3 passes pair
    (dy,0)+(dy,-1); 3 passes do (dy,+1) with zero weights on shifted rows.
  - Col-tiled matmuls (tile_position (0,32j), mode 128x32): 4 concurrent
    M=32 matmuls per pass, one per spatial tile, each writing ITS gate's
    slice so each PSUM bank ends up gate-pure:
      bank[g] = [128 part = 32ch x 4 spatial tiles, 512] of gate g.
    All pointwise LSTM math then runs on full 128-partition tiles.
  - c state persistent fp32 [128, 4096]; h written back (fp32 -> HBM out,
    bf16 -> next step's act tile via SBUF-SBUF DMA with layout shuffle).
"""

import os
import sys

import numpy as np

for _p in ("/opt/trn_rl_repo", "/root/.axon_site/_ro/trn_rl_repo"):
    if os.path.isdir(_p) and _p not in sys.path:
        sys.path.insert(0, _p)

import ml_dtypes  # noqa: E402
import concourse.bass as bass  # noqa: E402,F401
import concourse.mybir as mybir  # noqa: E402
from concourse import bacc, tile  # noqa: E402
from concourse.bass_utils import run_bass_kernel_spmd  # noqa: E402

F32 = mybir.dt.float32
BF16 = mybir.dt.bfloat16
AF = mybir.ActivationFunctionType

S, B, CIN, H, W = 16, 4, 32, 128, 128
CO = 32
HP, WP = H + 2, W + 2          # 130 x 130 padded plane
PADN = HP * WP                 # 16900
NSP = H * W                    # 16384
NT = 512                       # spatial positions per matmul tile (4 rows)
TPG = 4                        # tiles per group (col-tiled together)
GROUPS = NSP // (NT * TPG)     # 8 groups per step; group = 16 image rows
N_CORES = 8


def build_kernel(nc, tc, x_ap, w_ap, b_ap, y_ap, z_ap, n_steps):
    ctx_pools = []

    def pool(**kw):
        p = tc.tile_pool(**kw)
        ctx_pools.append(p)
        return p.__enter__()

    const = pool(name="const", bufs=1)
    tmp = pool(name="tmp", bufs=3)
    psum = pool(name="psum", bufs=8, space="PSUM")

    # Persistent tiles
    a0 = const.tile([128, PADN], BF16, tag="act0")
    a1 = const.tile([128, PADN], BF16, tag="act1")
    acts = [a0, a1]
    ctile = const.tile([128, GROUPS * NT], F32, tag="c")
    wsb = const.tile([128, 24 * 32], BF16, tag="w")
    bsb = const.tile([128, 4], F32, tag="bias")
    # per-step h (bf16) in padded row-chunks of 130: partition = 32*tile+ch,
    # free = 8 groups x 4 rows x 130.  hc chunk = [0, w0..w127, 0] (center
    # alignment), hs chunk = [0, 0, w0..w127] (shift alignment).  Pads are
    # zeroed once; whole 4-row blocks are then contiguous 520-elem runs, so
    # the write-back into the padded act planes is a 3-dim DMA per tile b.
    NTP = 4 * WP  # 520
    hc = const.tile([128, GROUPS * NTP], BF16, tag="hc")
    hs = const.tile([128, GROUPS * NTP], BF16, tag="hs")

    nc.sync.dma_start(wsb[:, :], w_ap)
    nc.sync.dma_start(bsb[:, :], b_ap)

    # --- one-time zero init ---
    for a in acts:
        ar = a.rearrange("p (r w) -> p r w", r=HP)
        nc.gpsimd.memset(a[:, 0:WP], 0.0)                       # pad row 0
        nc.gpsimd.memset(a[:, (HP - 1) * WP :], 0.0)            # pad row 129
        nc.gpsimd.memset(ar[:, :, 0:1], 0.0)                    # col 0 (unread, keep finite)
        nc.gpsimd.memset(ar[0:64, :, WP - 1 : WP], 0.0)         # center col 129
        nc.gpsimd.memset(ar[64:128, :, 1:2], 0.0)               # shift col 1
    # h regions of step-0 act buffer (h_0 = 0): DMA zeros from DRAM (cheaper
    # than big serialized memsets).  ctile needs no init: step 0 skips f*c.
    nc.sync.dma_start(a0[32:64, :], z_ap)
    nc.sync.dma_start(a0[96:128, :], z_ap)
    # pad columns of the h row-chunk buffers (never overwritten)
    hcb = hc.rearrange("p (k w) -> p k w", w=WP)
    hsb = hs.rearrange("p (k w) -> p k w", w=WP)
    nc.vector.memset(hcb[:, :, 0:1], 0.0)
    nc.vector.memset(hcb[:, :, WP - 1 : WP], 0.0)
    nc.gpsimd.memset(hsb[:, :, 0:2], 0.0)

    def load_x(t):
        # x arrives host-padded to the full 130x130 plane: both the center
        # copy and the +1-shifted copy are single contiguous runs per
        # partition (the shift picks up its zero border from the host pad).
        nc.sync.dma_start(acts[t % 2][0:32, :], x_ap[t])
        nc.sync.dma_start(acts[t % 2][64:96, 1:PADN], x_ap[t, :, 0 : PADN - 1])

    load_x(0)

    # h write-back: 4-row full-width (130) blocks, one 3-dim DMA per tile b.
    # Block (g, b) lands at padded plane rows 16g+4b+1 .. +5, cols 0..130.
    hcv = hc.rearrange("p (g f) -> p g f", g=GROUPS)
    hsv = hs.rearrange("p (g f) -> p g f", g=GROUPS)
    hcq = hc.rearrange("p (g q w) -> p g q w", g=GROUPS, q=4)
    hsq = hs.rearrange("p (g q w) -> p g q w", g=GROUPS, q=4)

    def h_writeback(a_nxt, g0, g1):
        # interior rows 1..129 of the padded plane = 32 blocks of (4 rows x 130)
        vc = a_nxt[32:64, WP : WP + H * WP].rearrange(
            "p (g b f) -> p g b f", g=GROUPS, b=TPG
        )
        vs = a_nxt[96:128, WP : WP + H * WP].rearrange(
            "p (g b f) -> p g b f", g=GROUPS, b=TPG
        )
        for b in range(TPG):
            nc.sync.dma_start(vc[:, g0:g1, b, :], hcv[32 * b : 32 * b + 32, g0:g1, :])
            nc.sync.dma_start(vs[:, g0:g1, b, :], hsv[32 * b : 32 * b + 32, g0:g1, :])

    for t in range(n_steps):
        a_cur = acts[t % 2]
        a_nxt = acts[(t + 1) % 2]
        ar_cur = a_cur.rearrange("p (r w) -> p r w", r=HP)
        if t + 1 < n_steps:
            load_x(t + 1)

        def pointwise(grp, zb):
            csl = ctile[:, grp * NT : (grp + 1) * NT]
            si = tmp.tile([128, NT], F32, tag="si", name=f"si{t}_{grp}")
            sf = tmp.tile([128, NT], F32, tag="sf", name=f"sf{t}_{grp}")
            so = tmp.tile([128, NT], F32, tag="so", name=f"so{t}_{grp}")
            tg = tmp.tile([128, NT], F32, tag="tg", name=f"tg{t}_{grp}")
            nc.scalar.activation(si[:, :], zb[0][:, :], AF.Sigmoid, bias=bsb[:, 0:1])
            if t > 0:
                nc.scalar.activation(sf[:, :], zb[1][:, :], AF.Sigmoid, bias=bsb[:, 1:2])
            nc.scalar.activation(so[:, :], zb[2][:, :], AF.Sigmoid, bias=bsb[:, 2:3])
            nc.scalar.activation(tg[:, :], zb[3][:, :], AF.Tanh, bias=bsb[:, 3:4])

            if t == 0:
                # c_{-1} = 0: c = sig(i)*tanh(g), no f*c term (ctile uninit)
                nc.vector.tensor_mul(csl, si[:, :], tg[:, :])
            else:
                t2 = tmp.tile([128, NT], F32, tag="t2", name=f"t2_{t}_{grp}")
                t3 = tmp.tile([128, NT], F32, tag="t3", name=f"t3_{t}_{grp}")
                # t3 only needs sf (2nd activation) - run it before t2 so the
                # DVE isn't blocked behind tanh_g on the critical chain
                nc.vector.tensor_mul(t3[:, :], sf[:, :], csl)
                nc.vector.tensor_mul(t2[:, :], si[:, :], tg[:, :])
                nc.vector.tensor_add(csl, t2[:, :], t3[:, :])

            tcn = tmp.tile([128, NT], F32, tag="tcn", name=f"tcn{t}_{grp}")
            nc.scalar.activation(tcn[:, :], csl, AF.Tanh)
            # h in bf16, written into the padded row-chunk buffers: hc serves
            # the y store + center write-back, hs the shifted write-back.
            soq = so[:, :].rearrange("p (q w) -> p q w", q=4)
            tcq = tcn[:, :].rearrange("p (q w) -> p q w", q=4)
            nc.vector.tensor_mul(hcq[:, grp, :, 1 : W + 1], soq, tcq)
            if t + 1 < n_steps:
                nc.vector.tensor_mul(hsq[:, grp, :, 2 : W + 2], soq, tcq)

        # Group pairs: adjacent groups' matmuls interleaved so same-weight
        # matmuls are consecutive (better PE pipelining).
        for pg in range(GROUPS // 2):
            grps = (2 * pg, 2 * pg + 1)
            zbs = [
                [
                    psum.tile([128, NT], F32, tag="z", name=f"z{t}_{grp}_{g}")
                    for g in range(4)
                ]
                for grp in grps
            ]
            for g in range(4):
                for p in range(6):
                    dy = (p % 3) - 1
                    dxo = 0 if p < 3 else 1
                    col = (g * 6 + p) * 32
                    lhsT = wsb[:, col : col + 32]
                    for j in range(TPG):
                        for gi, grp in enumerate(grps):
                            r0 = 16 * grp + 4 * j
                            rhs = ar_cur[
                                :, r0 + 1 + dy : r0 + 5 + dy, 1 + dxo : W + 1 + dxo
                            ]
                            nc.tensor.matmul(
                                zbs[gi][g][32 * j : 32 * j + 32, :],
                                lhsT,
                                rhs,
                                start=(p == 0),
                                stop=(p == 5),
                                skip_group_check=True,
                                tile_position=(0, 32 * j),
                            )
            pointwise(grps[0], zbs[0])
            pointwise(grps[1], zbs[1])
            if pg == 1:
                if t + 1 < n_steps:
                    h_writeback(a_nxt, 0, 4)
                nc.sync.dma_start(y_ap[t, :, 0 : 4 * NTP], hc[:, 0 : 4 * NTP])
            elif pg == 3:
                if t + 1 < n_steps:
                    h_writeback(a_nxt, 4, 8)
                nc.sync.dma_start(y_ap[t, :, 4 * NTP :], hc[:, 4 * NTP :])

    for p in reversed(ctx_pools):
        p.__exit__(None, None, None)


def build_program(n_steps=S):
    nc = bacc.Bacc(
        "TRN2",
        target_bir_lowering=False,
        debug=False,
        enable_asserts=False,
        num_devices=N_CORES,
    )
    x_d = nc.dram_tensor("x", [n_steps, CIN, PADN], BF16, kind="ExternalInput")
    w_d = nc.dram_tensor("w", [128, 24 * 32], BF16, kind="ExternalInput")
    b_d = nc.dram_tensor("bias", [128, 4], F32, kind="ExternalInput")
    # y in padded raw layout: [t, 32*tile+ch, group*520 + 130*q + (1+w)], bf16
    y_d = nc.dram_tensor(
        "y", [n_steps, 128, GROUPS * 4 * WP], BF16, kind="ExternalOutput"
    )
    z_d = nc.dram_tensor("z0", [32, PADN], BF16, kind="ExternalInput")
    with tile.TileContext(nc) as tc:
        build_kernel(
            nc, tc, x_d.ap(), w_d.ap(), b_d.ap(), y_d.ap(), z_d.ap(), n_steps
        )
    nc.compile()
    return nc


def pack_weights(Wd):
    """Wd [128, 64, 3, 3] f32 -> lhsT blocks [128, 24*32] bf16."""
    wp = np.zeros((128, 24, 32), np.float32)
    for g in range(4):
        Wg = Wd[g * 32 : (g + 1) * 32]  # [32(m), 64, 3, 3]
        for p in range(6):
            ky = (p % 3)  # dy + 1
            blk = wp[:, g * 6 + p, :]
            if p < 3:
                blk[0:32, :] = Wg[:, 0:32, ky, 1].T    # x, dx=0 via center rows
                blk[32:64, :] = Wg[:, 32:64, ky, 1].T  # h, dx=0
                blk[64:96, :] = Wg[:, 0:32, ky, 0].T   # x, dx=-1 via shifted rows
                blk[96:128, :] = Wg[:, 32:64, ky, 0].T
            else:
                blk[0:32, :] = Wg[:, 0:32, ky, 2].T    # x, dx=+1 via center rows
                blk[32:64, :] = Wg[:, 32:64, ky, 2].T
    return wp.reshape(128, 24 * 32).astype(ml_dtypes.bfloat16)


def pack_bias(bd):
    """bd [128] f32 -> [128, 4] f32 (partition p = 32*tile + ch)."""
    bp = np.zeros((128, 4), np.float32)
    for g in range(4):
        bp[:, g] = np.tile(bd[g * 32 : (g + 1) * 32], 4)
    return bp


def make_in_maps(x, Wf, bf, Wb, bb, n_steps=S):
    wpacks = [pack_weights(np.asarray(Wf, np.float32)),
              pack_weights(np.asarray(Wb, np.float32))]
    bpacks = [pack_bias(np.asarray(bf, np.float32)),
              pack_bias(np.asarray(bb, np.float32))]
    x = np.asarray(x, np.float32)
    in_maps = []
    for k in range(N_CORES):
        d, b = k // 4, k % 4
        xc = x[:n_steps, b] if d == 0 else x[::-1][:n_steps, b]
        xp = np.zeros((n_steps, CIN, HP, WP), ml_dtypes.bfloat16)
        xp[:, :, 1 : H + 1, 1 : W + 1] = xc
        in_maps.append(
            {
                "x": xp.reshape(n_steps, CIN, PADN),
                "w": wpacks[d],
                "bias": bpacks[d],
                "z0": np.zeros((32, PADN), ml_dtypes.bfloat16),
            }
        )
    return in_maps


_CACHED_NC = None


def unpack_y(yk):
    """[S, 128, 8*4*130] bf16 padded raw layout -> [S, CO, H, W] f32."""
    yk = np.asarray(yk, np.float32).reshape(S, TPG, CO, GROUPS, 4, WP)[..., 1 : W + 1]
    return np.ascontiguousarray(yk.transpose(0, 2, 3, 1, 4, 5)).reshape(S, CO, H, W)


def kernel(x, Wf, bf, Wb, bb):
    global _CACHED_NC
    if _CACHED_NC is None:
        _CACHED_NC = build_program(S)
    nc = _CACHED_NC
    in_maps = make_in_maps(x, Wf, bf, Wb, bb)
    res = run_bass_kernel_spmd(nc, in_maps, core_ids=list(range(N_CORES)))
    out = np.empty((S, B, 2 * CO, H, W), np.float32)
    for k in range(N_CORES):
        d, b = k // 4, k % 4
        out[:, b, d * CO : (d + 1) * CO] = unpack_y(res.results[k]["y"])
    return out


if __name__ == "__main__":
    import jax

    jax.config.update("jax_platforms", "cpu")
    rng = np.random.default_rng(0)
    x = rng.standard_normal((S, B, CIN, H, W), np.float32)
    Wf = (rng.standard_normal((128, 64, 3, 3)) * 0.05).astype(np.float32)
    Wb = (rng.standard_normal((128, 64, 3, 3)) * 0.05).astype(np.float32)
    bf = np.zeros(128, np.float32)
    bb = np.zeros(128, np.float32)
    y = kernel(x, Wf, bf, Wb, bb)
    print("out", y.shape, y.dtype)



# revision 26
# speedup vs baseline: 1.4801x; 1.0147x over previous
"""Bidirectional ConvLSTM block for Trainium2 (Bass/Tile), 8-core SPMD.

Problem: x [S=16, B=4, Cin=32, H=128, W=128] f32, Wf/Wb [128, 64, 3, 3],
bf/bb [128].  Output [S, B, 2*Co=64, H, W]: forward ConvLSTM hidden states
concat backward ConvLSTM (run on time-reversed x, not re-flipped).

Sharding: 8 independent recurrences = 2 directions x 4 batch elements.
Core k runs direction d=k//4 on batch b=k%4.  No cross-core communication.

Per-core kernel design:
  - SBUF "act" tile [128 part, 130*132] bf16 (ping/pong): zero-padded
    (H+2)x(W+4) spatial plane per channel.
      partitions  0-31 : x_t   centered at col 1+w   (pads 0, 129..131)
      partitions 32-63 : h_{t-1} centered at col 1+w
      partitions 64-95 : x_t   at col 2+w (shifted)  (pads 0,1, 130,131)
      partitions 96-127: h_{t-1} at col 2+w
    With the extra pad column, BOTH copies reach all 9 conv taps via rhs
    column offsets (center: 1+dx in 0..2, shift: 2+dx in 1..3).
  - Matmul schedule: per pair of groups (alpha, beta) and gate, 9 slots of
    two concurrent K=64 row-tiled matmuls: center rows (0-63) compute one
    tap for one group while shift rows (64-127) compute a tap for the
    OTHER group, writing different PSUM banks.  9 slots cover 9 taps for
    both groups -> zero wasted K rows (vs 6 passes of K=128 with 25%
    zero-weight rows).  Col tiling (0|64, 32j) runs the 4 spatial tiles
    of each half concurrently:
      bank[group][gate] = [128 part = 32ch x 4 spatial tiles, 512].
    All pointwise LSTM math runs on full 128-partition tiles.
  - c state persistent fp32 [128, 4096]; h (bf16) written into padded
    4-row chunk buffers hc/hs, which serve the y store (1 DMA per half
    step) and the batched write-back into the next act tile (one 3-dim
    SBUF->SBUF DMA per spatial tile b per half).
"""

import os
import sys

import numpy as np

for _p in ("/opt/trn_rl_repo", "/root/.axon_site/_ro/trn_rl_repo"):
    if os.path.isdir(_p) and _p not in sys.path:
        sys.path.insert(0, _p)

import ml_dtypes  # noqa: E402
import concourse.bass as bass  # noqa: E402,F401
import concourse.mybir as mybir  # noqa: E402
from concourse import bacc, tile  # noqa: E402
from concourse.bass_utils import run_bass_kernel_spmd  # noqa: E402

F32 = mybir.dt.float32
BF16 = mybir.dt.bfloat16
AF = mybir.ActivationFunctionType

S, B, CIN, H, W = 16, 4, 32, 128, 128
CO = 32
HP, WP = H + 2, W + 4          # 130 x 132 padded plane
PADN = HP * WP                 # 17160
NSP = H * W                    # 16384
NT = 512                       # spatial positions per matmul tile (4 rows)
NTP = 4 * WP                   # 528: padded 4-row chunk
TPG = 4                        # tiles per group (col-tiled together)
GROUPS = NSP // (NT * TPG)     # 8 groups per step; group = 16 image rows
N_CORES = 8

# 9 conv taps, row-major
TAPS = [(dy, dx) for dy in (-1, 0, 1) for dx in (-1, 0, 1)]

SCHEME = os.environ.get("BICLSTM_SCHEME", "9slot")  # "9slot" | "6pass"


def build_kernel(nc, tc, x_ap, w_ap, b_ap, y_ap, x0_ap, n_steps):
    ctx_pools = []

    def pool(**kw):
        p = tc.tile_pool(**kw)
        ctx_pools.append(p)
        return p.__enter__()

    const = pool(name="const", bufs=1)
    tmp = pool(name="tmp", bufs=3)
    psum = pool(name="psum", bufs=8, space="PSUM")

    # Persistent tiles
    a0 = const.tile([128, PADN], BF16, tag="act0")
    a1 = const.tile([128, PADN], BF16, tag="act1")
    acts = [a0, a1]
    ctile = const.tile([128, GROUPS * NT], F32, tag="c")
    wsb = const.tile([128, 4 * 9 * 32], BF16, tag="w")
    bsb = const.tile([128, 4], F32, tag="bias")
    hc = const.tile([128, GROUPS * NTP], BF16, tag="hc")
    hs = const.tile([128, GROUPS * NTP], BF16, tag="hs")

    nc.sync.dma_start(wsb[:, :], w_ap)
    nc.sync.dma_start(bsb[:, :], b_ap)
    # step-0 act tile fully host-prepared (x, shifted x, zero h, all pads)
    nc.sync.dma_start(a0[:, :], x0_ap)

    # --- one-time zero init (everything else is overwritten every step) ---
    # h pad rows 0/129 of the OTHER act tile (fold writes interior only)
    nc.gpsimd.memset(a1[32:64, 0:WP], 0.0)
    nc.gpsimd.memset(a1[32:64, (HP - 1) * WP :], 0.0)
    nc.gpsimd.memset(a1[96:128, 0:WP], 0.0)
    nc.gpsimd.memset(a1[96:128, (HP - 1) * WP :], 0.0)
    # pad columns of the h row-chunk buffers (never overwritten)
    hcb = hc.rearrange("p (k w) -> p k w", w=WP)
    hsb = hs.rearrange("p (k w) -> p k w", w=WP)
    nc.vector.memset(hcb[:, :, 0:1], 0.0)
    nc.vector.memset(hcb[:, :, W + 1 :], 0.0)
    nc.gpsimd.memset(hsb[:, :, 0:2], 0.0)
    nc.gpsimd.memset(hsb[:, :, W + 2 :], 0.0)

    def load_x(t):
        # x arrives host-padded; the shifted copy is the same stream offset
        # by one element (picks up its zero border from the host pad).
        nc.sync.dma_start(acts[t % 2][0:32, :], x_ap[t])
        nc.sync.dma_start(acts[t % 2][64:96, 1:PADN], x_ap[t, :, 0 : PADN - 1])

    # h write-back: 4-row full-width (132) blocks, one 3-dim DMA per tile b.
    # Block (g, b) lands at padded plane rows 16g+4b+1 .. +5, cols 0..132.
    hcv = hc.rearrange("p (g f) -> p g f", g=GROUPS)
    hsv = hs.rearrange("p (g f) -> p g f", g=GROUPS)
    hcq = hc.rearrange("p (g q w) -> p g q w", g=GROUPS, q=4)
    hsq = hs.rearrange("p (g q w) -> p g q w", g=GROUPS, q=4)

    def h_writeback(a_nxt, g0, g1):
        # interior rows 1..129 of the padded plane = 32 blocks of (4 x 132)
        vc = a_nxt[32:64, WP : WP + H * WP].rearrange(
            "p (g b f) -> p g b f", g=GROUPS, b=TPG
        )
        vs = a_nxt[96:128, WP : WP + H * WP].rearrange(
            "p (g b f) -> p g b f", g=GROUPS, b=TPG
        )
        for b in range(TPG):
            nc.sync.dma_start(vc[:, g0:g1, b, :], hcv[32 * b : 32 * b + 32, g0:g1, :])
            nc.sync.dma_start(vs[:, g0:g1, b, :], hsv[32 * b : 32 * b + 32, g0:g1, :])

    # Slot schedule per pair (alpha, beta): center rows serve one group's
    # tap while shift rows serve the other group's, into different banks.
    # 9 slots cover all 9 taps for both groups.
    #   type-1 k=0..4: center: alpha tap k   | shift: beta tap 4+k
    #   type-2 k=0..3: center: beta tap k    | shift: alpha tap 5+k
    def slot_schedule(ga, gb):
        return [(ga, k, gb, 4 + k) for k in range(5)] + [
            (gb, k, ga, 5 + k) for k in range(4)
        ]

    for t in range(n_steps):
        a_cur = acts[t % 2]
        a_nxt = acts[(t + 1) % 2]
        ar_cur = a_cur.rearrange("p (r w) -> p r w", r=HP)
        if t + 1 < n_steps:
            load_x(t + 1)

        def pointwise(grp, zb):
            csl = ctile[:, grp * NT : (grp + 1) * NT]
            si = tmp.tile([128, NT], F32, tag="si", name=f"si{t}_{grp}")
            so = tmp.tile([128, NT], F32, tag="so", name=f"so{t}_{grp}")
            tg = tmp.tile([128, NT], F32, tag="tg", name=f"tg{t}_{grp}")
            nc.scalar.activation(si[:, :], zb[0][:, :], AF.Sigmoid, bias=bsb[:, 0:1])
            if t > 0:
                sf = tmp.tile([128, NT], F32, tag="sf", name=f"sf{t}_{grp}")
                nc.scalar.activation(sf[:, :], zb[1][:, :], AF.Sigmoid, bias=bsb[:, 1:2])
            nc.scalar.activation(so[:, :], zb[2][:, :], AF.Sigmoid, bias=bsb[:, 2:3])
            nc.scalar.activation(tg[:, :], zb[3][:, :], AF.Tanh, bias=bsb[:, 3:4])

            if t == 0:
                # c_{-1} = 0: c = sig(i)*tanh(g), no f*c term (ctile uninit)
                nc.vector.tensor_mul(csl, si[:, :], tg[:, :])
            else:
                t2 = tmp.tile([128, NT], F32, tag="t2", name=f"t2_{t}_{grp}")
                t3 = tmp.tile([128, NT], F32, tag="t3", name=f"t3_{t}_{grp}")
                # t3 only needs sf (2nd activation) - run it before t2 so the
                # DVE isn't blocked behind tanh_g on the critical chain
                nc.vector.tensor_mul(t3[:, :], sf[:, :], csl)
                nc.vector.tensor_mul(t2[:, :], si[:, :], tg[:, :])
                nc.vector.tensor_add(csl, t2[:, :], t3[:, :])

            tcn = tmp.tile([128, NT], F32, tag="tcn", name=f"tcn{t}_{grp}")
            nc.scalar.activation(tcn[:, :], csl, AF.Tanh)
            # h in bf16, written into the padded row-chunk buffers: hc serves
            # the y store + center write-back, hs the shifted write-back.
            soq = so[:, :].rearrange("p (q w) -> p q w", q=4)
            tcq = tcn[:, :].rearrange("p (q w) -> p q w", q=4)
            nc.vector.tensor_mul(hcq[:, grp, :, 1 : W + 1], soq, tcq)
            if t + 1 < n_steps:
                nc.vector.tensor_mul(hsq[:, grp, :, 2 : W + 2], soq, tcq)

        for pg in range(GROUPS // 2):
            ga, gb = 2 * pg, 2 * pg + 1
            zbs = {
                grp: [
                    psum.tile([128, NT], F32, tag="z", name=f"z{t}_{grp}_{g}")
                    for g in range(4)
                ]
                for grp in (ga, gb)
            }
            if SCHEME == "6pass":
                for g in range(4):
                    for p in range(6):
                        dy = (p % 3) - 1
                        dxo = 0 if p < 3 else 1
                        col = (g * 6 + p) * 32
                        lhsT = wsb[:, col : col + 32]
                        for j in range(TPG):
                            for grp in (ga, gb):
                                r0 = 16 * grp + 4 * j
                                rhs = ar_cur[
                                    :, r0 + 1 + dy : r0 + 5 + dy, 1 + dxo : W + 1 + dxo
                                ]
                                nc.tensor.matmul(
                                    zbs[grp][g][32 * j : 32 * j + 32, :],
                                    lhsT,
                                    rhs,
                                    start=(p == 0),
                                    stop=(p == 5),
                                    skip_group_check=True,
                                    tile_position=(0, 32 * j),
                                )
                pointwise(ga, zbs[ga])
                pointwise(gb, zbs[gb])
                if pg == 1:
                    if t + 1 < n_steps:
                        h_writeback(a_nxt, 0, 4)
                    nc.sync.dma_start(y_ap[t, :, 0 : 4 * NTP], hc[:, 0 : 4 * NTP])
                elif pg == 3:
                    if t + 1 < n_steps:
                        h_writeback(a_nxt, 4, 8)
                    nc.sync.dma_start(y_ap[t, :, 4 * NTP :], hc[:, 4 * NTP :])
                continue
            slots = slot_schedule(ga, gb)
            for g in range(4):
                for si_, (cu, ct, su, st) in enumerate(slots):
                    first, last = si_ == 0, si_ == len(slots) - 1
                    dy, dx = TAPS[ct]
                    lhsT = wsb[0:64, (g * 9 + ct) * 32 : (g * 9 + ct) * 32 + 32]
                    for j in range(TPG):
                        r0 = 16 * cu + 4 * j
                        rhs = ar_cur[0:64, r0 + 1 + dy : r0 + 5 + dy, 1 + dx : 1 + dx + W]
                        nc.tensor.matmul(
                            zbs[cu][g][32 * j : 32 * j + 32, :],
                            lhsT,
                            rhs,
                            start=first,
                            stop=last,
                            skip_group_check=True,
                            tile_position=(0, 32 * j),
                        )
                    dy, dx = TAPS[st]
                    lhsT = wsb[64:128, (g * 9 + st) * 32 : (g * 9 + st) * 32 + 32]
                    for j in range(TPG):
                        r0 = 16 * su + 4 * j
                        rhs = ar_cur[
                            64:128, r0 + 1 + dy : r0 + 5 + dy, 2 + dx : 2 + dx + W
                        ]
                        nc.tensor.matmul(
                            zbs[su][g][32 * j : 32 * j + 32, :],
                            lhsT,
                            rhs,
                            start=first,
                            stop=last,
                            skip_group_check=True,
                            tile_position=(64, 32 * j),
                        )
            pointwise(ga, zbs[ga])
            pointwise(gb, zbs[gb])
            if pg == 1:
                if t + 1 < n_steps:
                    h_writeback(a_nxt, 0, 4)
                nc.sync.dma_start(y_ap[t, :, 0 : 4 * NTP], hc[:, 0 : 4 * NTP])
            elif pg == 3:
                if t + 1 < n_steps:
                    h_writeback(a_nxt, 4, 8)
                nc.sync.dma_start(y_ap[t, :, 4 * NTP :], hc[:, 4 * NTP :])

    for p in reversed(ctx_pools):
        p.__exit__(None, None, None)


def build_program(n_steps=S):
    nc = bacc.Bacc(
        "TRN2",
        target_bir_lowering=False,
        debug=False,
        enable_asserts=False,
        num_devices=N_CORES,
    )
    x_d = nc.dram_tensor("x", [n_steps, CIN, PADN], BF16, kind="ExternalInput")
    w_d = nc.dram_tensor("w", [128, 4 * 9 * 32], BF16, kind="ExternalInput")
    b_d = nc.dram_tensor("bias", [128, 4], F32, kind="ExternalInput")
    # y in padded raw layout: [t, 32*tile+ch, group*528 + 132*q + (1+w)], bf16
    y_d = nc.dram_tensor(
        "y", [n_steps, 128, GROUPS * NTP], BF16, kind="ExternalOutput"
    )
    x0_d = nc.dram_tensor("x0", [128, PADN], BF16, kind="ExternalInput")
    with tile.TileContext(nc) as tc:
        build_kernel(
            nc, tc, x_d.ap(), w_d.ap(), b_d.ap(), y_d.ap(), x0_d.ap(), n_steps
        )
    nc.compile()
    return nc


def pack_weights(Wd):
    """Wd [128, 64, 3, 3] f32 -> lhsT blocks [128, 4*9*32] bf16.

    Block (g, tap): [64, 32] with rows 0-31 = x weights, 32-63 = h weights;
    duplicated on partitions 0-63 (center use) and 64-127 (shift use).
    """
    wp = np.zeros((128, 36, 32), np.float32)
    for g in range(4):
        Wg = Wd[g * 32 : (g + 1) * 32]  # [32(m), 64, 3, 3]
        if SCHEME == "6pass":
            for p in range(6):
                ky = p % 3
                blk = wp[:, g * 6 + p, :]
                if p < 3:
                    blk[0:32, :] = Wg[:, 0:32, ky, 1].T
                    blk[32:64, :] = Wg[:, 32:64, ky, 1].T
                    blk[64:96, :] = Wg[:, 0:32, ky, 0].T
                    blk[96:128, :] = Wg[:, 32:64, ky, 0].T
                else:
                    blk[0:32, :] = Wg[:, 0:32, ky, 2].T
                    blk[32:64, :] = Wg[:, 32:64, ky, 2].T
            continue
        for ti, (dy, dx) in enumerate(TAPS):
            ky, kx = dy + 1, dx + 1
            blk = wp[:, g * 9 + ti, :]
            blk[0:32, :] = Wg[:, 0:32, ky, kx].T
            blk[32:64, :] = Wg[:, 32:64, ky, kx].T
            blk[64:96, :] = Wg[:, 0:32, ky, kx].T
            blk[96:128, :] = Wg[:, 32:64, ky, kx].T
    return wp.reshape(128, 36 * 32).astype(ml_dtypes.bfloat16)


def pack_bias(bd):
    """bd [128] f32 -> [128, 4] f32 (partition p = 32*tile + ch)."""
    bp = np.zeros((128, 4), np.float32)
    for g in range(4):
        bp[:, g] = np.tile(bd[g * 32 : (g + 1) * 32], 4)
    return bp


def make_in_maps(x, Wf, bf, Wb, bb, n_steps=S):
    wpacks = [pack_weights(np.asarray(Wf, np.float32)),
              pack_weights(np.asarray(Wb, np.float32))]
    bpacks = [pack_bias(np.asarray(bf, np.float32)),
              pack_bias(np.asarray(bb, np.float32))]
    x = np.asarray(x, np.float32)
    in_maps = []
    for k in range(N_CORES):
        d, b = k // 4, k % 4
        xc = x[:n_steps, b] if d == 0 else x[::-1][:n_steps, b]
        xp = np.zeros((n_steps, CIN, HP, WP), ml_dtypes.bfloat16)
        xp[:, :, 1 : H + 1, 1 : W + 1] = xc
        xp = xp.reshape(n_steps, CIN, PADN)
        # step-0 full act tile: x / zero-h / shifted x / zero-h, pads included
        x0 = np.zeros((128, PADN), ml_dtypes.bfloat16)
        x0[0:32] = xp[0]
        x0[64:96].reshape(-1)[1:] = xp[0].reshape(-1)[:-1]
        in_maps.append(
            {
                "x": xp,
                "w": wpacks[d],
                "bias": bpacks[d],
                "x0": x0,
            }
        )
    return in_maps


_CACHED_NC = None


def unpack_y(yk):
    """[S, 128, 8*4*132] bf16 padded raw layout -> [S, CO, H, W] f32."""
    yk = np.asarray(yk, np.float32).reshape(S, TPG, CO, GROUPS, 4, WP)[..., 1 : W + 1]
    return np.ascontiguousarray(yk.transpose(0, 2, 3, 1, 4, 5)).reshape(S, CO, H, W)


def kernel(x, Wf, bf, Wb, bb):
    global _CACHED_NC
    if _CACHED_NC is None:
        _CACHED_NC = build_program(S)
    nc = _CACHED_NC
    in_maps = make_in_maps(x, Wf, bf, Wb, bb)
    res = run_bass_kernel_spmd(nc, in_maps, core_ids=list(range(N_CORES)))
    out = np.empty((S, B, 2 * CO, H, W), np.float32)
    for k in range(N_CORES):
        d, b = k // 4, k % 4
        out[:, b, d * CO : (d + 1) * CO] = unpack_y(res.results[k]["y"])
    return out


if __name__ == "__main__":
    import jax

    jax.config.update("jax_platforms", "cpu")
    rng = np.random.default_rng(0)
    x = rng.standard_normal((S, B, CIN, H, W), np.float32)
    Wf = (rng.standard_normal((128, 64, 3, 3)) * 0.05).astype(np.float32)
    Wb = (rng.standard_normal((128, 64, 3, 3)) * 0.05).astype(np.float32)
    bf = np.zeros(128, np.float32)
    bb = np.zeros(128, np.float32)
    y = kernel(x, Wf, bf, Wb, bb)
    print("out", y.shape, y.dtype)
